# revision 1
# baseline (speedup 1.0000x reference)
# DigitCaps dynamic-routing kernel for 8 Trainium2 NeuronCores.
#
# Sharding: the prev-layer node axis P=6272 is split across the 8 cores
# (784 nodes each).  Per core, both W-slices and u-slices live in SBUF in
# bf16 for the whole kernel; every routing sweep recomputes u_hat tiles
# on the PE from SBUF instead of streaming a 514MB u_hat through HBM.
# The only cross-core traffic is three small AllReduces of the per-core
# partial sums s_raw[n,b,o] (+ softmax denominators Z[n,b]).
#
# Layout glossary (per core, local p in [0,784)):
#   NB layout: partitions = (ns, bs) = 8 caps x 16 batch  (per n-group g, b-half h)
#   P  layout: partitions = local p (7 chunks of 128, last chunk 16 valid)
#   w2   [5g][128=(ns,o)][784p][8i]   rhs of the wv matmul (streamed from HBM)
#   wp2  [128=p][7ch][40n][128=(i,o)] lhsT of the s matmul
#   up2  [128=p][7ch][8i][32b]        rhs of the s matmul (iter 1) / cu fold input
#   urep [2h][128=(ns,bs)][784p][8i]  u replicated over ns, for the a-pass fold
import os
import numpy as np
import ml_dtypes

import concourse.bass as bass
import concourse.bacc as bacc
import concourse.tile as tile
import concourse.mybir as mybir
from concourse.bass_utils import run_bass_kernel_spmd

BF16 = mybir.dt.bfloat16
F32 = mybir.dt.float32
AX = mybir.AxisListType
ALU = mybir.AluOpType
ACTF = mybir.ActivationFunctionType

N, P, I, O, B = 40, 6272, 8, 16, 32
NC = 8
PL = P // NC          # 784 local nodes
NG = 5                # n-groups of 8
BH = 2                # b-halves of 16
CH = 7                # p-chunks of 128 (last has 16 valid rows)
PPAD = CH * 128       # 896
NB_TILES = NG * BH    # 10 (g, h) tiles; tile t = 2*g + h
CCLEN = O * N * B + N * B  # 20480 s_raw + 1280 Z


def _build_program(for_sim=False):
    nc = bacc.Bacc("TRN2", target_bir_lowering=False, debug=False)

    w2 = nc.dram_tensor("w2", [NG, 128, PL, I], BF16, kind="ExternalInput")
    wp2 = nc.dram_tensor("wp2", [128, CH, N, 128], BF16, kind="ExternalInput")
    up2 = nc.dram_tensor("up2", [128, CH, I, B], BF16, kind="ExternalInput")
    urep = nc.dram_tensor("urep", [BH, 128, PL, I], BF16, kind="ExternalInput")
    bdmask = nc.dram_tensor("bdmask", [128, 128], BF16, kind="ExternalInput")
    selio = nc.dram_tensor("selio", [128, I, 16], F32, kind="ExternalInput")
    id128b = nc.dram_tensor("id128b", [128, 128], BF16, kind="ExternalInput")
    id16f = nc.dram_tensor("id16f", [16, 16], F32, kind="ExternalInput")
    vout = nc.dram_tensor("vout", [N, B, O], F32, kind="ExternalOutput")

    with tile.TileContext(nc) as tc:
        with (
            tc.tile_pool(name="res", bufs=1) as res,       # whole-kernel residents
            tc.tile_pool(name="w2s", bufs=3) as w2s,       # streamed w2 pieces
            tc.tile_pool(name="work", bufs=3) as work,     # wv/t chunk tiles
            tc.tile_pool(name="atile", bufs=2) as atile,   # a / b2 tiles
            tc.tile_pool(name="cupool", bufs=2) as cupool,
            tc.tile_pool(name="sm", bufs=1) as sm,         # small per-tile stats
            tc.tile_pool(name="ps_s", bufs=1, space="PSUM") as ps_s,
            tc.tile_pool(name="ps_wv", bufs=2, space="PSUM") as ps_wv,
            tc.tile_pool(name="ps_m", bufs=2, space="PSUM") as ps_m,
            tc.tile_pool(name="dram", bufs=2, space="DRAM") as dram,
        ):
            # ---- residents ----
            sb_wp2 = res.tile([128, CH, N, 128], BF16)
            nc.sync.dma_start(out=sb_wp2, in_=wp2[:])
            sb_up2 = res.tile([128, CH, I, B], BF16)
            nc.sync.dma_start(out=sb_up2, in_=up2[:])
            sb_urep0 = res.tile([128, PL, I], BF16)
            nc.sync.dma_start(out=sb_urep0, in_=urep[0])
            sb_urep1 = res.tile([128, PL, I], BF16)
            nc.sync.dma_start(out=sb_urep1, in_=urep[1])
            sb_urep = [sb_urep0, sb_urep1]
            sb_mask = res.tile([128, 128], BF16)
            nc.sync.dma_start(out=sb_mask, in_=bdmask[:])
            sb_sel = res.tile([128, I, 16], F32)
            nc.sync.dma_start(out=sb_sel, in_=selio[:])
            sb_id128b = res.tile([128, 128], BF16)
            nc.sync.dma_start(out=sb_id128b, in_=id128b[:])
            sb_id16f = res.tile([16, 16], F32)
            nc.sync.dma_start(out=sb_id16f, in_=id16f[:])

            a1_dram = dram.tile([NB_TILES, 128, PL], F32, tag="a1", bufs=1)

            sb_eP2 = res.tile([128, CH, N, B], BF16)
            nc.vector.memset(sb_eP2, 0.0)
            sb_Z = res.tile([128, NB_TILES], F32)
            # iteration-1 "Z": AllReduce over 8 cores must sum to P (uniform c)
            nc.vector.memset(sb_Z, float(P) / NC)
            # s_raw columns in (g, h, ns, bs) order: col 128*(2g+h) + 16*ns + bs
            sb_sraw = res.tile([16, NG, BH, 8, 16], F32)
            sb_sglob = res.tile([16, N * B], F32)
            sb_Zg = res.tile([128, NB_TILES], F32)
            sb_vT = res.tile([16, N * B], BF16)
            sb_vT8 = res.tile([128, N * B], BF16)
            sb_fac = res.tile([128, NB_TILES], F32)
            sb_ss = res.tile([128, NB_TILES], F32)

            junk_sb = res.tile([128, 1], F32)

            def pe_touch(*aps):
                """Tiny matmuls that consume pending semaphores on the PE
                queue (the lowered LDWEIGHTS has a single sync-wait slot, so
                real matmuls may carry at most one fresh dependency)."""
                for ap in aps:
                    j = ps_s.tile([1, 1], F32, tag="ps_s", name="jnk")
                    nc.tensor.matmul(j, ap, ap, start=True, stop=True)

            def act_touch(ap):
                nc.scalar.copy(out=junk_sb[0 : ap.shape[0], :], in_=ap)

            def s_pass(it):
                """Partial s_raw[n,b,o] = sum_{p local,i} cu * W, AllReduce,
                squash.  it=1 uses cu = u (uniform c), else cu = eP2 * up2."""
                for jb in range(NG * 2):
                    g_, nhalf = jb // 2, jb % 2
                    psum_s = ps_s.tile([128, 4, 256], F32, tag="ps_s", bufs=2)
                    for nn in range(4):
                        n = 4 * jb + nn
                        if it == 1:
                            cu = sb_up2
                        else:
                            cu = cupool.tile([128, CH, I, B], BF16, tag="cu")
                            e_sl = bass.AP(
                                tensor=sb_eP2.tensor,
                                offset=sb_eP2.offset + n * B,
                                ap=[sb_eP2.ap[0], [N * B, CH], [0, I], [1, B]],
                            )
                            nc.vector.tensor_tensor(
                                out=cu, in0=sb_up2, in1=e_sl, op=ALU.mult
                            )
                        cu_flat = cu.rearrange("q c i b -> q c (i b)")
                        for ch in range(CH):
                            nc.tensor.matmul(
                                psum_s[:, nn, :],
                                sb_wp2[:, ch, n, :],
                                cu_flat[:, ch, :],
                                start=(ch == 0),
                                stop=(ch == CH - 1),
                            )
                    # extract s[o,n,b] = sum_i psum[(i,o), nn, (i,b)]:
                    # copy the whole block to SBUF, then 8 accumulating
                    # selector matmuls: sel[:,i,:] keeps only rows (i,o)
                    # while the rhs free-offset slides to the i-th b-slice.
                    Ssb = sm.tile([128, I, 4, B], F32, tag="S2", bufs=2)
                    nc.scalar.copy(
                        out=Ssb.rearrange("q i n b -> q n i b"),
                        in_=psum_s.rearrange("q n (i b) -> q n i b", i=I),
                    )
                    Ssb_flat = Ssb.rearrange("q i n b -> q i (n b)")
                    sel_ps = ps_m.tile([16, 4, B], F32, tag="m")
                    for i in range(I):
                        nc.tensor.matmul(
                            sel_ps,
                            sb_sel[:, i, :],
                            Ssb_flat[:, i, :],
                            start=(i == 0),
                            stop=(i == I - 1),
                        )
                    nc.scalar.copy(
                        out=sb_sraw[:, g_, :, 4 * nhalf : 4 * nhalf + 4, :]
                        .rearrange("o h n b -> o n h b"),
                        in_=sel_ps.rearrange("o n (h b) -> o n h b", h=BH),
                    )

                # ---- AllReduce (s_raw ++ Z) ----
                cc_in = dram.tile([CCLEN], F32, tag="cc_in")
                cc_out = dram.tile([CCLEN], F32, tag="cc_out")
                nc.gpsimd.dma_start(out=cc_in[0 : O * N * B], in_=sb_sraw)
                nc.gpsimd.dma_start(out=cc_in[O * N * B :], in_=sb_Z)
                if for_sim:
                    nc.gpsimd.dma_start(out=cc_out, in_=cc_in)
                else:
                    nc.gpsimd.collective_compute(
                        "AllReduce",
                        ALU.add,
                        replica_groups=[list(range(NC))],
                        ins=[cc_in.opt()],
                        outs=[cc_out.opt()],
                    )
                nc.gpsimd.dma_start(out=sb_sglob, in_=cc_out[0 : O * N * B])
                nc.gpsimd.dma_start(out=sb_Zg, in_=cc_out[O * N * B :])

                # ---- squash per (g,h) tile ----
                for t in range(NB_TILES):
                    g, h = t // BH, t % BH
                    s_sl = sb_sglob[:, 128 * t : 128 * (t + 1)]
                    sq_ps = ps_m.tile([128, 16], F32, tag="m")
                    nc.tensor.transpose(sq_ps, s_sl, sb_id16f)
                    sq_sb = sm.tile([128, 16], F32, tag="sqs")
                    nc.scalar.copy(out=sq_sb, in_=sq_ps)
                    sq2 = sm.tile([128, 16], F32, tag="sq2")
                    nc.vector.tensor_tensor(out=sq2, in0=sq_sb, in1=sq_sb, op=ALU.mult)
                    nc.vector.tensor_reduce(
                        out=sb_ss[:, t : t + 1], in_=sq2, axis=AX.X, op=ALU.add
                    )
                    if it == 3:
                        z2 = sm.tile([128, 1], F32, tag="z2")
                        nc.vector.tensor_tensor(
                            out=z2, in0=sb_Zg[:, t : t + 1], in1=sb_Zg[:, t : t + 1],
                            op=ALU.mult,
                        )
                        den = sm.tile([128, 1], F32, tag="den")
                        nc.vector.tensor_tensor(
                            out=den, in0=z2, in1=sb_ss[:, t : t + 1], op=ALU.add
                        )
                        rec = sm.tile([128, 1], F32, tag="rec")
                        nc.vector.reciprocal(out=rec, in_=den)
                        rss = sm.tile([128, 1], F32, tag="rss")
                        nc.scalar.sqrt(out=rss, in_=sb_ss[:, t : t + 1])
                        nc.vector.tensor_tensor(
                            out=sb_fac[:, t : t + 1], in0=rss, in1=rec, op=ALU.mult
                        )
                        vt = sm.tile([128, 16], F32, tag="vt")
                        nc.scalar.mul(out=vt, in_=sq_sb, mul=sb_fac[:, t : t + 1])
                        nc.sync.dma_start(
                            out=vout[8 * g : 8 * g + 8, 16 * h : 16 * h + 16, :],
                            in_=vt,
                        )
                if it < 3:
                    # batched squash scalars for all 10 tiles in one op each
                    z2a = sm.tile([128, NB_TILES], F32, tag="z2")
                    nc.vector.tensor_tensor(out=z2a, in0=sb_Zg, in1=sb_Zg, op=ALU.mult)
                    dena = sm.tile([128, NB_TILES], F32, tag="den")
                    nc.vector.tensor_tensor(out=dena, in0=z2a, in1=sb_ss, op=ALU.add)
                    reca = sm.tile([128, NB_TILES], F32, tag="rec")
                    nc.vector.reciprocal(out=reca, in_=dena)
                    rssa = sm.tile([128, NB_TILES], F32, tag="rss")
                    nc.scalar.sqrt(out=rssa, in_=sb_ss)
                    nc.vector.tensor_tensor(
                        out=sb_fac, in0=rssa, in1=reca, op=ALU.mult
                    )
                    # unnormalized v^T (the squash factor is applied later,
                    # per-partition, inside the wv PSUM->SBUF copy)
                    nc.scalar.copy(out=sb_vT, in_=sb_sglob)
                    for r in range(8):
                        nc.gpsimd.dma_start(
                            out=sb_vT8[16 * r : 16 * r + 16, :], in_=sb_vT
                        )

            def a_pass(it):
                """a[n,b,p] = u_hat . v for every local p; also fuses the
                bridge for the next s-pass (exp, Z, transpose into eP2)."""
                for g in range(NG):
                    bds, ats = [], []
                    for h in range(BH):
                        bd = sm.tile([128, 128], BF16, tag="bd", bufs=2)
                        t_ = 2 * g + h
                        nc.vector.tensor_tensor(
                            out=bd,
                            in0=sb_vT8[:, 128 * t_ : 128 * (t_ + 1)],
                            in1=sb_mask,
                            op=ALU.mult,
                        )
                        bds.append(bd)
                        ats.append(atile.tile([128, PL], F32, tag="a", name=f"at{h}"))

                    # stream w2[g] in 4 pieces of up to 256 nodes each
                    for pc in range(4):
                        pn = 256 if pc < 3 else 16
                        w2p = w2s.tile([128, 256, I], BF16, tag="w2p")
                        nc.sync.dma_start(
                            out=w2p[:, :pn, :],
                            in_=w2[g, :, 256 * pc : 256 * pc + pn, :],
                        )
                        for h in range(BH):
                            t = 2 * g + h
                            wv_sb = None
                            for sck in range(4 if pc < 3 else 1):
                                pw = 64 if pc < 3 else 16
                                F = pw * I
                                off = 256 * pc + 64 * sck  # global node offset
                                wv_ps = ps_wv.tile([128, 512], F32, tag="wv")
                                nc.tensor.matmul(
                                    wv_ps[:, :F],
                                    bds[h],
                                    w2p.rearrange("q p i -> q (p i)")[
                                        :, 512 * sck : 512 * sck + F
                                    ],
                                    start=True,
                                    stop=True,
                                )
                                # pair two 64-node chunks into one 128-node
                                # DVE fold to amortize per-op overhead
                                if wv_sb is None:
                                    wv_sb = work.tile([128, 128, I], BF16, tag="wvs")
                                half = sck % 2
                                nc.scalar.mul(
                                    out=wv_sb[:, 64 * half : 64 * half + pw, :],
                                    in_=wv_ps.rearrange("q (p i) -> q p i", i=I)[
                                        :, :pw, :
                                    ],
                                    mul=sb_fac[:, t : t + 1],
                                )
                                if pc < 3 and half == 0:
                                    continue  # wait for the second half
                                mw = 128 if pc < 3 else 16  # merged width
                                moff = off - 64 * half
                                ts_ = work.tile([128, 128, I], BF16, tag="ts")
                                nc.vector.tensor_tensor(
                                    out=ts_[:, :mw, :],
                                    in0=wv_sb[:, :mw, :],
                                    in1=sb_urep[h][:, moff : moff + mw, :],
                                    op=ALU.mult,
                                )
                                r1 = work.tile([128, 128, 4], BF16, tag="r1")
                                nc.vector.tensor_tensor(
                                    out=r1[:, :mw, :], in0=ts_[:, :mw, 0:4],
                                    in1=ts_[:, :mw, 4:8], op=ALU.add,
                                )
                                r2 = work.tile([128, 128, 2], BF16, tag="r2")
                                nc.vector.tensor_tensor(
                                    out=r2[:, :mw, :], in0=r1[:, :mw, 0:2],
                                    in1=r1[:, :mw, 2:4], op=ALU.add,
                                )
                                nc.vector.tensor_tensor(
                                    out=ats[h][:, moff : moff + mw],
                                    in0=r2[:, :mw, 0], in1=r2[:, :mw, 1], op=ALU.add,
                                )
                                wv_sb = None
                    # ---- bridge to next s-pass ----
                    for h in range(BH):
                        t = 2 * g + h
                        at = ats[h]
                        if it == 1:
                            nc.gpsimd.dma_start(out=a1_dram[t], in_=at)
                            bt = at
                        else:
                            a1b = atile.tile([128, PL], F32, tag="a1b")
                            nc.gpsimd.dma_start(out=a1b, in_=a1_dram[t])
                            bt = atile.tile([128, PL], F32, tag="b2")
                            nc.vector.tensor_tensor(out=bt, in0=at, in1=a1b, op=ALU.add)
                        e_nb = work.tile([128, PL], BF16, tag="enb")
                        nc.scalar.activation(
                            out=e_nb, in_=bt, func=ACTF.Exp,
                            accum_out=sb_Z[:, t : t + 1],
                        )
                        for ch in range(CH):
                            pw = 128 if ch < 6 else 16
                            eT_ps = ps_m.tile([128, 128], BF16, tag="m")
                            nc.tensor.transpose(
                                eT_ps[:pw, :],
                                e_nb[:, 128 * ch : 128 * ch + pw],
                                sb_id128b,
                            )
                            nc.scalar.copy(
                                out=sb_eP2[:pw, ch, 8 * g : 8 * g + 8,
                                           16 * h : 16 * h + 16],
                                in_=eT_ps.rearrange("p (n b) -> p n b", n=8)[:pw],
                            )

            s_pass(1)
            a_pass(1)
            s_pass(2)
            a_pass(2)
            s_pass(3)

    nc.finalize()
    return nc


_CACHE = {}


def _prep_inputs(u, W):
    """Per-core host-side relayout (not part of HW time)."""
    bf = ml_dtypes.bfloat16
    maps = []
    for c in range(NC):
        sl = slice(PL * c, PL * (c + 1))
        Wc = np.ascontiguousarray(W[:, sl])          # [40, 784, 8, 16] f32
        uc = np.ascontiguousarray(u[:, sl])          # [32, 784, 8] f32
        w2 = (
            Wc.reshape(NG, 8, PL, I, O)
            .transpose(0, 1, 4, 2, 3)
            .reshape(NG, 128, PL, I)
        )
        Wp = np.zeros((N, PPAD, I, O), np.float32)
        Wp[:, :PL] = Wc
        wp2 = Wp.reshape(N, CH, 128, 128).transpose(2, 1, 0, 3)
        Up = np.zeros((B, PPAD, I), np.float32)
        Up[:, :PL] = uc
        up2 = Up.reshape(B, CH, 128, I).transpose(2, 1, 3, 0)
        ur = np.broadcast_to(
            uc.reshape(1, BH, 16, PL, I), (8, BH, 16, PL, I)
        ).transpose(1, 0, 2, 3, 4).reshape(BH, 128, PL, I)
        bdm = np.zeros((128, 128), np.float32)
        for ns in range(8):
            bdm[ns * 16 : ns * 16 + 16, ns * 16 : ns * 16 + 16] = 1.0
        sel = np.zeros((128, I, 16), np.float32)
        for i in range(I):
            sel[16 * i : 16 * i + 16, i] = np.eye(16, dtype=np.float32)
        maps.append(
            {
                "w2": np.ascontiguousarray(w2).astype(bf),
                "wp2": np.ascontiguousarray(wp2).astype(bf),
                "up2": np.ascontiguousarray(up2).astype(bf),
                "urep": np.ascontiguousarray(ur).astype(bf),
                "bdmask": bdm.astype(bf),
                "selio": sel,
                "id128b": np.eye(128, dtype=np.float32).astype(bf),
                "id16f": np.eye(16, dtype=np.float32),
            }
        )
    return maps


def kernel(u, W):
    u = np.asarray(u, np.float32)
    W = np.asarray(W, np.float32)
    if "nc" not in _CACHE:
        _CACHE["nc"] = _build_program()
    nc = _CACHE["nc"]
    in_maps = _prep_inputs(u, W)
    res = run_bass_kernel_spmd(
        nc, in_maps, core_ids=list(range(NC)),
        trace=bool(int(os.environ.get("KERNEL_TRACE", "0"))),
    )
    _CACHE["last_result"] = res
    return res.results[0]["vout"]



# revision 51
# speedup vs baseline: 1.3929x; 1.3929x over previous
# DigitCaps dynamic-routing kernel for 8 Trainium2 NeuronCores.
#
# Sharding: the prev-layer node axis P=6272 is split across the 8 cores
# (784 nodes each).  Per core, both W-slices and u-slices live in SBUF in
# bf16 for the whole kernel; every routing sweep recomputes u_hat tiles
# on the PE from SBUF instead of streaming a 514MB u_hat through HBM.
# The only cross-core traffic is three small AllReduces of the per-core
# partial sums s_raw[n,b,o] (+ softmax denominators Z[n,b]).
#
# Layout glossary (per core, local p in [0,784)):
#   NB layout: partitions = (ns, bs) = 8 caps x 16 batch  (per n-group g, b-half h)
#   P  layout: partitions = local p (7 chunks of 128, last chunk 16 valid)
#   w2   [5g][128=(ns,o)][784p][8i]   rhs of the wv matmul (streamed from HBM)
#   wp2  [128=p][7ch][40n][128=(i,o)] lhsT of the s matmul
#   up2  [128=p][7ch][8i][32b]        rhs of the s matmul (iter 1) / cu fold input
#   urep [2h][128=(ns,bs)][784p][8i]  u replicated over ns, for the a-pass fold
#
# Perf notes (cost-model driven):
#  - v is pre-scaled by the squash factor before the broadcast, so the
#    a-pass wv matmuls write the final scaled values straight into bf16
#    PSUM and the DVE fold (tensor_tensor at 2x_1p) reads PSUM directly
#    -- no per-chunk Activation-engine scaled copy.
#  - a1 (iteration-1 agreements) stays resident in SBUF (bf16).
#  - each s-pass (it>=2) is emitted fused under the previous a-pass's
#    n-group loop so PE matmuls and DVE folds interleave.
#  - the wp2 resident load is split into 10 n-slices so the first s-pass
#    matmuls start as soon as the first slice lands.
import os
import numpy as np
import ml_dtypes

import concourse.bass as bass
import concourse.bacc as bacc
import concourse.tile as tile
import concourse.mybir as mybir
from concourse.bass_utils import run_bass_kernel_spmd

BF16 = mybir.dt.bfloat16
F32 = mybir.dt.float32
AX = mybir.AxisListType
ALU = mybir.AluOpType
ACTF = mybir.ActivationFunctionType

N, P, I, O, B = 40, 6272, 8, 16, 32
NC = 8
PL = P // NC          # 784 local nodes
NG = 5                # n-groups of 8
BH = 2                # b-halves of 16
CH = 7                # p-chunks of 128 (last chunk 16 valid rows)
PPAD = CH * 128       # 896
NB_TILES = NG * BH    # 10 (g, h) tiles; tile t = 2*g + h
CCLEN = O * N * B + N * B  # 20480 s_raw + 1280 Z

# tiles whose a-pass fold reads PSUM directly on DVE (route B); the rest
# drain PSUM through an Activation-engine copy first (route A).  Tuned
# against the cost-model timeline to balance DVE vs Activation busy time.
ROUTE_B_TILES = frozenset()
R1_POOL_TILES = frozenset()  # tiles whose r1 tree-add runs on GPSIMD/Pool
R2_POOL = False              # Pool cannot run TensorScalarPtr on real HW
W_S = 0   # PE warmers per s-block half
W_A = 0   # PE warmers per a-pass chunk


def _build_program(for_sim=False):
    nc = bacc.Bacc("TRN2", target_bir_lowering=False, debug=False)

    w2 = nc.dram_tensor("w2", [NG, 128, PL, I], BF16, kind="ExternalInput")
    wp2 = nc.dram_tensor("wp2", [128, CH, N, 128], BF16, kind="ExternalInput")
    up2 = nc.dram_tensor("up2", [128, CH, I, B], BF16, kind="ExternalInput")
    urep = nc.dram_tensor("urep", [BH, 128, PL, I], BF16, kind="ExternalInput")
    bdmask = nc.dram_tensor("bdmask", [128, 128], BF16, kind="ExternalInput")
    selio = nc.dram_tensor("selio", [128, I, 16], BF16, kind="ExternalInput")
    id128b = nc.dram_tensor("id128b", [128, 128], BF16, kind="ExternalInput")
    id16f = nc.dram_tensor("id16f", [16, 16], F32, kind="ExternalInput")
    rep16 = nc.dram_tensor("rep16", [16, 128], BF16, kind="ExternalInput")
    wq = nc.dram_tensor("wq", [NG, 128, 49, 128], BF16, kind="ExternalInput")
    up3 = nc.dram_tensor("up3", [128, 49, B], BF16, kind="ExternalInput")
    selio2 = nc.dram_tensor("selio2", [128, 8, 16], BF16, kind="ExternalInput")
    vout = nc.dram_tensor("vout", [N, B, O], F32, kind="ExternalOutput")

    with tile.TileContext(nc) as tc:
        with (
            tc.tile_pool(name="res", bufs=1) as res,       # whole-kernel residents
            tc.tile_pool(name="w2s", bufs=3) as w2s,       # streamed w2 pieces
            tc.tile_pool(name="work", bufs=3) as work,     # fold chunk tiles
            tc.tile_pool(name="atile", bufs=4) as atile,   # a / b2 tiles
            tc.tile_pool(name="cupool", bufs=2) as cupool,
            tc.tile_pool(name="sm", bufs=1) as sm,         # small per-tile stats
            tc.tile_pool(name="ps_s", bufs=2, space="PSUM") as ps_s,
            tc.tile_pool(name="ps_wv", bufs=2, space="PSUM") as ps_wv,
            tc.tile_pool(name="ps_m", bufs=1, space="PSUM") as ps_m,
            tc.tile_pool(name="dram", bufs=2, space="DRAM") as dram,
        ):
            # ---- residents; ordered so s-pass-1 can start early ----
            sb_up3 = res.tile([128, 49, B], BF16)
            nc.sync.dma_start(out=sb_up3, in_=up3[:])
            sb_up2 = res.tile([128, CH, I, B], BF16)
            nc.sync.dma_start(out=sb_up2, in_=up2[:])
            sb_sel = res.tile([128, I, 16], BF16)
            nc.sync.dma_start(out=sb_sel, in_=selio[:])
            sb_w2r = res.tile([128, NG, PL, I], BF16)
            w2r_flat = sb_w2r.rearrange("q g p i -> q g (p i)")
            sb_id16f = res.tile([16, 16], F32)
            nc.sync.dma_start(out=sb_id16f, in_=id16f[:])
            sb_rep16 = res.tile([16, 128], BF16)
            nc.sync.dma_start(out=sb_rep16, in_=rep16[:])
            sb_sel2 = res.tile([128, 8, 16], BF16)
            nc.sync.dma_start(out=sb_sel2, in_=selio2[:])
            sb_mask = res.tile([128, 128], BF16)
            nc.sync.dma_start(out=sb_mask, in_=bdmask[:])
            sb_id128b = res.tile([128, 128], BF16)
            nc.sync.dma_start(out=sb_id128b, in_=id128b[:])
            sb_urep0 = res.tile([128, PL, I], BF16)
            sb_urep1 = res.tile([128, PL, I], BF16)
            sb_urep = [sb_urep0, sb_urep1]

            sb_eP2 = res.tile([128, CH, N, B], BF16)
            # pad rows (p>=784 and ch6 partitions 16:128) must be finite:
            # cu multiplies them by up2's zero padding.
            nc.gpsimd.memset(sb_eP2, 0.0)
            sb_Z = res.tile([128, NB_TILES], F32)
            # iteration-1 "Z": AllReduce over 8 cores must sum to P (uniform c)
            nc.vector.memset(sb_Z, float(P) / NC)
            # s_raw columns in (g, h, ns, bs) order: col 128*(2g+h) + 16*ns + bs
            sb_sraw = res.tile([16, NG, BH, 8, 16], F32)
            sb_sglob = res.tile([16, N * B], F32)
            sb_Zg = res.tile([128, NB_TILES], F32)
            sb_vT = res.tile([16, N * B], BF16)      # squash-scaled v^T
            sb_vT8 = res.tile([128, N * B], BF16)
            sb_fac = res.tile([128, NB_TILES], F32)
            sb_ss = res.tile([128, NB_TILES], F32)
            sb_sq = res.tile([128, NB_TILES, 16], F32)   # s in NB layout
            sb_a1 = res.tile([128, NB_TILES, PL], BF16)  # iteration-1 agreements

            up2_flat = sb_up2.rearrange("q c i b -> q (c i b)")

            def warm(k):
                """Junk matmuls that keep the PE p-state ramped through
                gaps (the cost model halves the clock after any idle)."""
                for _ in range(k):
                    wj = ps_m.tile([16, 128], F32, tag="eT")
                    nc.tensor.matmul(
                        wj, up2_flat[:, 0:16], up2_flat[:, 0:128],
                        start=True, stop=True,
                    )

            pending_sel = []

            def s_jb(it, jb):
                """Partial s_raw for the 4 caps of block jb:
                s_raw[n,b,o] = sum_{p local,i} cu * W with cu = e * u
                (it=1 uses cu = u: uniform c)."""
                g_, nhalf = jb // 2, jb % 2
                wps = w2s.tile([128, CH, 4, 128], BF16, tag="wps")
                nc.sync.dma_start(
                    out=wps, in_=wp2[:, :, 4 * jb : 4 * jb + 4, :]
                )
                for half in range(2):
                    psum_s = ps_s.tile([128, 2, 256], F32, tag="ps_s")
                    for nn in range(2):
                        n = 4 * jb + 2 * half + nn
                        if it == 1:
                            cu = sb_up2
                        else:
                            cu = cupool.tile([128, CH, I, B], BF16, tag="cu")
                            e_sl = bass.AP(
                                tensor=sb_eP2.tensor,
                                offset=sb_eP2.offset + n * B,
                                ap=[sb_eP2.ap[0], [N * B, CH], [0, I], [1, B]],
                            )
                            nc.vector.tensor_tensor(
                                out=cu, in0=sb_up2, in1=e_sl, op=ALU.mult
                            )
                        cu_flat = cu.rearrange("q c i b -> q c (i b)")
                        for ch in range(CH):
                            nc.tensor.matmul(
                                psum_s[:, nn, :],
                                wps[:, ch, 2 * half + nn, :],
                                cu_flat[:, ch, :],
                                start=(ch == 0),
                                stop=(ch == CH - 1),
                            )
                    # drain psum to SBUF; the selector extraction is deferred
                    # (lag-1) so it never head-blocks the PE queue between
                    # consecutive jb matmul groups.
                    Ssb = sm.tile([128, I, 2, B], BF16, tag="S2", bufs=3)
                    nc.scalar.copy(
                        out=Ssb.rearrange("q i n b -> q n i b"),
                        in_=psum_s.rearrange("q n (i b) -> q n i b", i=I),
                    )
                    pending_sel.append((Ssb, g_, 4 * nhalf + 2 * half))
                while len(pending_sel) > 2:
                    emit_sel(*pending_sel.pop(0))

            def emit_sel(Ssb, g_, no):
                """s[o,n,b] = sum_i Ssb[(i,o), n, (i,b)] via 8 accumulating
                selector matmuls, then the sraw copy."""
                Ssb_flat = Ssb.rearrange("q i n b -> q i (n b)")
                sel_ps = ps_m.tile([16, 2, B], F32, tag="m")
                for i in range(I):
                    nc.tensor.matmul(
                        sel_ps,
                        sb_sel[:, i, :],
                        Ssb_flat[:, i, :],
                        start=(i == 0),
                        stop=(i == I - 1),
                    )
                nc.scalar.copy(
                    out=sb_sraw[:, g_, :, no : no + 2, :]
                    .rearrange("o h n b -> o n h b"),
                    in_=sel_ps.rearrange("o n (h b) -> o n h b", h=BH),
                )

            def flush_sels():
                while pending_sel:
                    emit_sel(*pending_sel.pop(0))

            def allreduce_squash(it):
                """AllReduce (s_raw ++ Z), then squash; for it<3 also emit the
                pre-scaled v^T (fac folded in) and its 8-way broadcast.
                Everything is batched across the 10 (g,h) tiles to keep this
                inter-pass bridge short."""
                cc_in = dram.tile([CCLEN], F32, tag="cc_in")
                cc_out = dram.tile([CCLEN], F32, tag="cc_out")
                nc.scalar.dma_start(out=cc_in[0 : O * N * B], in_=sb_sraw)
                nc.gpsimd.dma_start(out=cc_in[O * N * B :], in_=sb_Z)
                # dummy sqrt: pulls the Sqrt act-table load into the AR wait
                junk = sm.tile([128, 1], F32, tag="jnk")
                nc.scalar.sqrt(out=junk, in_=sb_fac[:, 0:1])
                if for_sim:
                    nc.gpsimd.dma_start(out=cc_out, in_=cc_in)
                else:
                    nc.gpsimd.collective_compute(
                        "AllReduce",
                        ALU.add,
                        replica_groups=[list(range(NC))],
                        ins=[cc_in.opt()],
                        outs=[cc_out.opt()],
                    )
                nc.scalar.dma_start(out=sb_sglob, in_=cc_out[0 : O * N * B])
                nc.gpsimd.dma_start(out=sb_Zg, in_=cc_out[O * N * B :])

                # squash runs in two batches: tiles 0-1 first (they gate
                # group 0 of the next a-pass), then tiles 2-9
                def squash_batch(t0, t1):
                    nt = t1 - t0
                    sqa_ps = ps_m.tile([128, NB_TILES, 16], F32, tag="m")
                    for t in range(t0, t1):
                        nc.tensor.transpose(
                            sqa_ps[:, t, :],
                            sb_sglob[:, 128 * t : 128 * (t + 1)],
                            sb_id16f,
                        )
                    sq2 = sm.tile([128, NB_TILES, 16], F32, tag="sq2")
                    nc.scalar.square(
                        out=sq2[:, t0:t1, :], in_=sqa_ps[:, t0:t1, :]
                    )
                    nc.scalar.copy(
                        out=sb_sq[:, t0:t1, :], in_=sqa_ps[:, t0:t1, :]
                    )
                    nc.vector.tensor_reduce(
                        out=bass.AP(
                            tensor=sb_ss.tensor,
                            offset=sb_ss.offset + t0,
                            ap=[sb_ss.ap[0], [1, nt], [1, 1]],
                        ),
                        in_=sq2[:, t0:t1, :], axis=AX.X, op=ALU.add,
                    )
                    # fac = sqrt(ss) / (Z^2 + ss)
                    z2a = sm.tile([128, NB_TILES], F32, tag="z2")
                    nc.vector.tensor_tensor(
                        out=z2a[:, t0:t1], in0=sb_Zg[:, t0:t1],
                        in1=sb_Zg[:, t0:t1], op=ALU.mult,
                    )
                    dena = sm.tile([128, NB_TILES], F32, tag="den")
                    nc.vector.tensor_tensor(
                        out=dena[:, t0:t1], in0=z2a[:, t0:t1],
                        in1=sb_ss[:, t0:t1], op=ALU.add,
                    )
                    reca = sm.tile([128, NB_TILES], F32, tag="rec")
                    nc.vector.reciprocal(out=reca[:, t0:t1], in_=dena[:, t0:t1])
                    rssa = sm.tile([128, NB_TILES], F32, tag="rss")
                    nc.scalar.sqrt(out=rssa[:, t0:t1], in_=sb_ss[:, t0:t1])
                    nc.vector.tensor_tensor(
                        out=sb_fac[:, t0:t1], in0=rssa[:, t0:t1],
                        in1=reca[:, t0:t1], op=ALU.mult,
                    )

                fac_bc0 = bass.AP(
                    tensor=sb_fac.tensor,
                    offset=sb_fac.offset,
                    ap=[sb_fac.ap[0], [1, NB_TILES], [0, 16]],
                )
                vtb = sm.tile(
                    [128, NB_TILES, 16], F32 if it == 3 else BF16, tag="vtb"
                )

                def v_batch(t0, t1):
                    fac_bc = bass.AP(
                        tensor=sb_fac.tensor,
                        offset=sb_fac.offset + t0,
                        ap=[sb_fac.ap[0], [1, t1 - t0], [0, 16]],
                    )
                    nc.vector.tensor_tensor(
                        out=vtb[:, t0:t1, :], in0=sb_sq[:, t0:t1, :],
                        in1=fac_bc, op=ALU.mult,
                    )
                    if it == 3:
                        for t in range(t0, t1):
                            g, h = t // BH, t % BH
                            nc.sync.dma_start(
                                out=vout[8 * g : 8 * g + 8,
                                         16 * h : 16 * h + 16, :],
                                in_=vtb[:, t, :],
                            )
                        return
                    # vT8[(r,o), nb] = vtb[nb, o]: per 4-tile group,
                    # transpose vtb -> vT (PSUM), drain, then one
                    # replication matmul per tile (rep16 x vT) + drain
                    for c0 in range(t0, t1, 4):
                        c1 = min(c0 + 4, t1)
                        vtT_ps = ps_m.tile([16, 4, 128], BF16, tag="eT")
                        for j in range(c1 - c0):
                            nc.tensor.transpose(
                                vtT_ps[:, j, :], vtb[:, c0 + j, :], sb_id128b
                            )
                        nc.scalar.copy(
                            out=sb_vT[:, 128 * c0 : 128 * c1],
                            in_=vtT_ps[:, : c1 - c0, :]
                            .rearrange("o t c -> o (t c)"),
                        )
                        for t in range(c0, c1):
                            v8_ps = ps_m.tile([128, 128], F32, tag="m")
                            nc.tensor.matmul(
                                v8_ps, sb_rep16,
                                sb_vT[:, 128 * t : 128 * (t + 1)],
                                start=True, stop=True,
                            )
                            nc.scalar.copy(
                                out=sb_vT8[:, 128 * t : 128 * (t + 1)],
                                in_=v8_ps,
                            )

                squash_batch(0, 2)
                v_batch(0, 2)
                squash_batch(2, NB_TILES)
                v_batch(2, NB_TILES)
                if it == 3:
                    return
                # dummy exp: preload the Exp act-table before the a-pass
                junk2 = sm.tile([128, 1], BF16, tag="jnk2")
                nc.scalar.activation(out=junk2, in_=sb_fac[:, 0:1], func=ACTF.Exp)

            def a_group(it, g):
                """a[n,b,p] = u_hat . v for group g's two (g,h) tiles, the
                bridge into eP2 for the next s-pass, and (fused) the next
                s-pass's two jb blocks for this group's caps."""
                bds, ats = [], []
                for h in range(BH):
                    t_ = 2 * g + h
                    bd = sm.tile([128, 128], BF16, tag="bd", bufs=2)
                    nc.vector.tensor_tensor(
                        out=bd,
                        in0=sb_vT8[:, 128 * t_ : 128 * (t_ + 1)],
                        in1=sb_mask,
                        op=ALU.mult,
                    )
                    bds.append(bd)
                    if it == 1:
                        ats.append(sb_a1[:, t_, :])
                    else:
                        ats.append(atile.tile([128, PL], BF16, tag="a", name=f"at{h}"))

                # stream w2[g] in 4 pieces of up to 256 nodes, fold in
                # 128-node chunks.  Two drain routes, balanced across tiles:
                #   A: Activation copies PSUM f32 -> SBUF bf16, DVE fold at 2x
                #   B: DVE fold reads PSUM f32 directly (full rate, no Act)
                for pc in range(4):
                    pn = 256 if pc < 3 else 16
                    off = 256 * pc
                    for h in range(BH):
                        t_ = 2 * g + h
                        route_b = t_ in ROUTE_B_TILES
                        for sub in range(2 if pc < 3 else 1):
                            cn = 128 if pc < 3 else 16
                            coff = off + 128 * sub
                            wv_ps = ps_wv.tile([128, 128, I], F32, tag="wv")
                            wv_flat = wv_ps.rearrange("q p i -> q (p i)")
                            for sck in range(2 if pc < 3 else 1):
                                F = (64 if pc < 3 else 16) * I
                                nc.tensor.matmul(
                                    wv_flat[:, 512 * sck : 512 * sck + F],
                                    bds[h],
                                    w2r_flat[
                                        :, g,
                                        I * coff + 512 * sck :
                                        I * coff + 512 * sck + F,
                                    ],
                                    start=True,
                                    stop=True,
                                )
                            warm(W_A)
                            if route_b:
                                wv_in = wv_ps
                            else:
                                wv_sb = work.tile([128, 128, I], BF16, tag="wvs")
                                nc.scalar.copy(
                                    out=wv_sb[:, :cn, :], in_=wv_ps[:, :cn, :]
                                )
                                wv_in = wv_sb
                            ts_ = work.tile([128, 128, I], BF16, tag="ts")
                            nc.vector.tensor_tensor(
                                out=ts_[:, :cn, :],
                                in0=wv_in[:, :cn, :],
                                in1=sb_urep[h][:, coff : coff + cn, :],
                                op=ALU.mult,
                            )
                            r1 = work.tile([128, 128, 4], BF16, tag="r1")
                            if t_ in R1_POOL_TILES:
                                nc.gpsimd.scalar_tensor_tensor(
                                    out=r1[:, :cn, :], in0=ts_[:, :cn, 0:4],
                                    scalar=1.0, in1=ts_[:, :cn, 4:8],
                                    op0=ALU.mult, op1=ALU.add,
                                )
                            else:
                                nc.vector.tensor_tensor(
                                    out=r1[:, :cn, :], in0=ts_[:, :cn, 0:4],
                                    in1=ts_[:, :cn, 4:8], op=ALU.add,
                                )
                            r2 = work.tile([128, 128, 2], BF16, tag="r2")
                            if R2_POOL:
                                nc.gpsimd.scalar_tensor_tensor(
                                    out=r2[:, :cn, :], in0=r1[:, :cn, 0:2],
                                    scalar=1.0, in1=r1[:, :cn, 2:4],
                                    op0=ALU.mult, op1=ALU.add,
                                )
                                nc.gpsimd.scalar_tensor_tensor(
                                    out=ats[h][:, coff : coff + cn],
                                    in0=r2[:, :cn, 0], scalar=1.0,
                                    in1=r2[:, :cn, 1],
                                    op0=ALU.mult, op1=ALU.add,
                                )
                            else:
                                nc.vector.tensor_tensor(
                                    out=r2[:, :cn, :], in0=r1[:, :cn, 0:2],
                                    in1=r1[:, :cn, 2:4], op=ALU.add,
                                )
                                nc.vector.tensor_tensor(
                                    out=ats[h][:, coff : coff + cn],
                                    in0=r2[:, :cn, 0], in1=r2[:, :cn, 1],
                                    op=ALU.add,
                                )

                return ats

            def a_bridge(it, g, ats):
                # ---- bridge to next s-pass: exp + transpose into eP2 ----
                for h in range(BH):
                    t = 2 * g + h
                    if it == 1:
                        bt = sb_a1[:, t, :]
                    else:
                        bt = atile.tile([128, PL], BF16, tag="b2", bufs=2)
                        nc.vector.tensor_tensor(
                            out=bt, in0=ats[h], in1=sb_a1[:, t, :], op=ALU.add
                        )
                    e_nb = work.tile([128, PL], BF16, tag="enb")
                    nc.scalar.activation(
                        out=e_nb, in_=bt, func=ACTF.Exp,
                        accum_out=sb_Z[:, t : t + 1],
                    )
                    # 7 PE transposes batched into two PSUM tiles, three
                    # Activation copies (ch6 separate: only 16 valid rows).
                    for cg, nch in ((0, 4), (4, 2)):
                        eT_ps = ps_m.tile([128, 4, 128], BF16, tag="eT")
                        for j in range(nch):
                            ch = cg + j
                            nc.tensor.transpose(
                                eT_ps[:, j, :],
                                e_nb[:, 128 * ch : 128 * (ch + 1)],
                                sb_id128b,
                            )
                        nc.scalar.copy(
                            out=sb_eP2[:, cg : cg + nch, 8 * g : 8 * g + 8,
                                       16 * h : 16 * h + 16],
                            in_=eT_ps[:, :nch, :]
                            .rearrange("p c (n b) -> p c n b", n=8),
                        )
                    eT6 = ps_m.tile([128, 4, 128], BF16, tag="eT")
                    nc.tensor.transpose(
                        eT6[:16, 0, :], e_nb[:, 768 : 768 + 16], sb_id128b
                    )
                    nc.scalar.copy(
                        out=sb_eP2[:16, 6, 8 * g : 8 * g + 8,
                                   16 * h : 16 * h + 16],
                        in_=eT6[:16, 0, :].rearrange("p (n b) -> p n b", n=8),
                    )



            # ---- schedule ----
            # s-pass 1: c is uniform, so the rhs (u) is shared across all
            # capsules -- contract (p,i) jointly with (n,o)-batched weights:
            # 49 accumulating 32-wide matmuls per n-group.
            ps1 = ps_s.tile([128, NG, B], F32, tag="ps_s")
            for g_ in range(NG):
                for half_ in range(2):
                    c0, c1 = (0, 25) if half_ == 0 else (25, 49)
                    wqp = w2s.tile([128, 25, 128], BF16, tag="wps")
                    nc.sync.dma_start(
                        out=wqp[:, : c1 - c0, :], in_=wq[g_, :, c0:c1, :]
                    )
                    for c_ in range(c0, c1):
                        nc.tensor.matmul(
                            ps1[:, g_, :],
                            wqp[:, c_ - c0, :],
                            sb_up3[:, c_, :],
                            start=(c_ == 0),
                            stop=(c_ == 48),
                        )
                pg = sm.tile([128, B], BF16, tag="pg", bufs=2)
                nc.scalar.copy(out=pg, in_=ps1[:, g_, :])
                for j_ in range(4):
                    sel_ps = ps_m.tile([16, 2, B], F32, tag="m")
                    for nn_ in range(2):
                        nc.tensor.matmul(
                            sel_ps[:, nn_, :],
                            sb_sel2[:, 2 * j_ + nn_, :],
                            pg,
                            start=True,
                            stop=True,
                        )
                    nc.scalar.copy(
                        out=sb_sraw[:, g_, :, 2 * j_ : 2 * j_ + 2, :]
                        .rearrange("o h n b -> o n h b"),
                        in_=sel_ps.rearrange("o n (h b) -> o n h b", h=BH),
                    )
            for q_ in range(4):
                nc.sync.dma_start(
                    out=sb_w2r[:, 0, 196 * q_ : 196 * (q_ + 1), :],
                    in_=w2[0, :, 196 * q_ : 196 * (q_ + 1), :],
                )
            for q_ in range(2):
                nc.sync.dma_start(
                    out=sb_urep0[:, 392 * q_ : 392 * (q_ + 1), :],
                    in_=urep[0, :, 392 * q_ : 392 * (q_ + 1), :],
                )
                nc.sync.dma_start(
                    out=sb_urep1[:, 392 * q_ : 392 * (q_ + 1), :],
                    in_=urep[1, :, 392 * q_ : 392 * (q_ + 1), :],
                )
            flush_sels()
            allreduce_squash(1)
            for g_ in range(1, NG):
                for q_ in range(4):
                    nc.sync.dma_start(
                        out=sb_w2r[:, g_, 196 * q_ : 196 * (q_ + 1), :],
                        in_=w2[g_, :, 196 * q_ : 196 * (q_ + 1), :],
                    )
            prev = None
            for g in range(NG):
                ats_g = a_group(1, g)
                if prev is not None:
                    a_bridge(1, g - 1, prev)
                    if g >= 2:
                        s_jb(2, 2 * (g - 2))
                        s_jb(2, 2 * (g - 2) + 1)
                prev = ats_g
            a_bridge(1, NG - 1, prev)
            for jb in (2 * (NG - 2), 2 * (NG - 2) + 1, 2 * (NG - 1),
                       2 * (NG - 1) + 1):
                s_jb(2, jb)
            flush_sels()
            allreduce_squash(2)
            prev = None
            for g in range(NG):
                ats_g = a_group(2, g)
                if prev is not None:
                    a_bridge(2, g - 1, prev)
                    if g >= 2:
                        s_jb(3, 2 * (g - 2))
                        s_jb(3, 2 * (g - 2) + 1)
                prev = ats_g
            a_bridge(2, NG - 1, prev)
            for jb in (2 * (NG - 2), 2 * (NG - 2) + 1, 2 * (NG - 1),
                       2 * (NG - 1) + 1):
                s_jb(3, jb)
            flush_sels()
            allreduce_squash(3)

    nc.finalize()
    return nc


_CACHE = {}


def _prep_inputs(u, W):
    """Per-core host-side relayout (not part of HW time)."""
    bf = ml_dtypes.bfloat16
    maps = []
    for c in range(NC):
        sl = slice(PL * c, PL * (c + 1))
        Wc = np.ascontiguousarray(W[:, sl])          # [40, 784, 8, 16] f32
        uc = np.ascontiguousarray(u[:, sl])          # [32, 784, 8] f32
        w2 = (
            Wc.reshape(NG, 8, PL, I, O)
            .transpose(0, 1, 4, 2, 3)
            .reshape(NG, 128, PL, I)
        )
        Wp = np.zeros((N, PPAD, I, O), np.float32)
        Wp[:, :PL] = Wc
        wp2 = Wp.reshape(N, CH, 128, 128).transpose(2, 1, 0, 3)
        Up = np.zeros((B, PPAD, I), np.float32)
        Up[:, :PL] = uc
        up2 = Up.reshape(B, CH, 128, I).transpose(2, 1, 3, 0)
        ur = np.broadcast_to(
            uc.reshape(1, BH, 16, PL, I), (8, BH, 16, PL, I)
        ).transpose(1, 0, 2, 3, 4).reshape(BH, 128, PL, I)
        bdm = np.zeros((128, 128), np.float32)
        for ns in range(8):
            bdm[ns * 16 : ns * 16 + 16, ns * 16 : ns * 16 + 16] = 1.0
        sel = np.zeros((128, I, 16), np.float32)
        for i in range(I):
            sel[16 * i : 16 * i + 16, i] = np.eye(16, dtype=np.float32)
        # wq[g, (pp,i), c, (n8,o)] = Wc[8g+n8, 16c+pp, i, o]
        wqh = (
            Wc.reshape(NG, 8, 49, 16, I, O)
            .transpose(0, 3, 4, 2, 1, 5)
            .reshape(NG, 128, 49, 128)
        )
        # up3[(pp,i), c, b] = uc[b, 16c+pp, i]
        u3 = (
            uc.reshape(B, 49, 16, I)
            .transpose(2, 3, 1, 0)
            .reshape(128, 49, B)
        )
        sel2 = np.zeros((128, 8, 16), np.float32)
        for cap in range(8):
            for o in range(16):
                sel2[16 * cap + o, cap, o] = 1.0
        maps.append(
            {
                "w2": np.ascontiguousarray(w2).astype(bf),
                "wp2": np.ascontiguousarray(wp2).astype(bf),
                "up2": np.ascontiguousarray(up2).astype(bf),
                "urep": np.ascontiguousarray(ur).astype(bf),
                "bdmask": bdm.astype(bf),
                "selio": sel.astype(bf),
                "id128b": np.eye(128, dtype=np.float32).astype(bf),
                "id16f": np.eye(16, dtype=np.float32),
                "rep16": np.tile(np.eye(16, dtype=np.float32), (1, 8)).astype(bf),
                "wq": np.ascontiguousarray(wqh).astype(bf),
                "up3": np.ascontiguousarray(u3).astype(bf),
                "selio2": sel2.astype(bf),
            }
        )
    return maps


def kernel(u, W):
    u = np.asarray(u, np.float32)
    W = np.asarray(W, np.float32)
    if "nc" not in _CACHE:
        _CACHE["nc"] = _build_program()
    nc = _CACHE["nc"]
    in_maps = _prep_inputs(u, W)
    res = run_bass_kernel_spmd(
        nc, in_maps, core_ids=list(range(NC)),
        trace=bool(int(os.environ.get("KERNEL_TRACE", "0"))),
    )
    _CACHE["last_result"] = res
    return res.results[0]["vout"]


# revision 55
# speedup vs baseline: 1.4024x; 1.0068x over previous
# DigitCaps dynamic-routing kernel for 8 Trainium2 NeuronCores.
#
# Sharding: the prev-layer node axis P=6272 is split across the 8 cores
# (784 nodes each).  Per core, both W-slices and u-slices live in SBUF in
# bf16 for the whole kernel; every routing sweep recomputes u_hat tiles
# on the PE from SBUF instead of streaming a 514MB u_hat through HBM.
# The only cross-core traffic is three small AllReduces of the per-core
# partial sums s_raw[n,b,o] (+ softmax denominators Z[n,b]).
#
# Layout glossary (per core, local p in [0,784)):
#   NB layout: partitions = (ns, bs) = 8 caps x 16 batch  (per n-group g, b-half h)
#   P  layout: partitions = local p (7 chunks of 128, last chunk 16 valid)
#   w2   [5g][128=(ns,o)][784p][8i]   rhs of the wv matmul (streamed from HBM)
#   wp2  [128=p][7ch][40n][128=(i,o)] lhsT of the s matmul
#   up2  [128=p][7ch][8i][32b]        rhs of the s matmul (iter 1) / cu fold input
#   urep [2h][128=(ns,bs)][784p][8i]  u replicated over ns, for the a-pass fold
#
# Perf notes (cost-model driven):
#  - v is pre-scaled by the squash factor before the broadcast, so the
#    a-pass wv matmuls write the final scaled values straight into bf16
#    PSUM and the DVE fold (tensor_tensor at 2x_1p) reads PSUM directly
#    -- no per-chunk Activation-engine scaled copy.
#  - a1 (iteration-1 agreements) stays resident in SBUF (bf16).
#  - each s-pass (it>=2) is emitted fused under the previous a-pass's
#    n-group loop so PE matmuls and DVE folds interleave.
#  - the wp2 resident load is split into 10 n-slices so the first s-pass
#    matmuls start as soon as the first slice lands.
import os
import numpy as np
import ml_dtypes

import concourse.bass as bass
import concourse.bacc as bacc
import concourse.tile as tile
import concourse.mybir as mybir
from concourse.bass_utils import run_bass_kernel_spmd

BF16 = mybir.dt.bfloat16
F32 = mybir.dt.float32
AX = mybir.AxisListType
ALU = mybir.AluOpType
ACTF = mybir.ActivationFunctionType

N, P, I, O, B = 40, 6272, 8, 16, 32
NC = 8
PL = P // NC          # 784 local nodes
NG = 5                # n-groups of 8
BH = 2                # b-halves of 16
CH = 7                # p-chunks of 128 (last chunk 16 valid rows)
PPAD = CH * 128       # 896
NB_TILES = NG * BH    # 10 (g, h) tiles; tile t = 2*g + h
CCLEN = O * N * B + N * B  # 20480 s_raw + 1280 Z

# tiles whose a-pass fold reads PSUM directly on DVE (route B); the rest
# drain PSUM through an Activation-engine copy first (route A).  Tuned
# against the cost-model timeline to balance DVE vs Activation busy time.
ROUTE_B_TILES = frozenset()
R1_POOL_TILES = frozenset()  # tiles whose r1 tree-add runs on GPSIMD/Pool
R2_POOL = False              # Pool cannot run TensorScalarPtr on real HW
W_S = 0   # PE warmers per s-block half
W_A = 0   # PE warmers per a-pass chunk


def _build_program(for_sim=False):
    nc = bacc.Bacc("TRN2", target_bir_lowering=False, debug=False)

    w2 = nc.dram_tensor("w2", [NG, 128, PL, I], BF16, kind="ExternalInput")
    wp2 = nc.dram_tensor("wp2", [128, CH, N, 128], BF16, kind="ExternalInput")
    up2 = nc.dram_tensor("up2", [128, CH, I, B], BF16, kind="ExternalInput")
    urep = nc.dram_tensor("urep", [BH, 128, PL, I], BF16, kind="ExternalInput")
    bdmask = nc.dram_tensor("bdmask", [128, 128], BF16, kind="ExternalInput")
    selio = nc.dram_tensor("selio", [128, I, 16], BF16, kind="ExternalInput")
    id128b = nc.dram_tensor("id128b", [128, 128], BF16, kind="ExternalInput")
    id16f = nc.dram_tensor("id16f", [16, 16], F32, kind="ExternalInput")
    rep16 = nc.dram_tensor("rep16", [16, 128], BF16, kind="ExternalInput")
    wq = nc.dram_tensor("wq", [NG, 128, 49, 128], BF16, kind="ExternalInput")
    up3 = nc.dram_tensor("up3", [128, 49, B], BF16, kind="ExternalInput")
    selio2 = nc.dram_tensor("selio2", [128, 8, 16], BF16, kind="ExternalInput")
    vout = nc.dram_tensor("vout", [N, B, O], F32, kind="ExternalOutput")

    with tile.TileContext(nc) as tc:
        with (
            tc.tile_pool(name="res", bufs=1) as res,       # whole-kernel residents
            tc.tile_pool(name="w2s", bufs=3) as w2s,       # streamed w2 pieces
            tc.tile_pool(name="work", bufs=3) as work,     # fold chunk tiles
            tc.tile_pool(name="atile", bufs=4) as atile,   # a / b2 tiles
            tc.tile_pool(name="cupool", bufs=2) as cupool,
            tc.tile_pool(name="sm", bufs=1) as sm,         # small per-tile stats
            tc.tile_pool(name="ps_s", bufs=2, space="PSUM") as ps_s,
            tc.tile_pool(name="ps_wv", bufs=2, space="PSUM") as ps_wv,
            tc.tile_pool(name="ps_m", bufs=1, space="PSUM") as ps_m,
            tc.tile_pool(name="dram", bufs=2, space="DRAM") as dram,
        ):
            # ---- residents; ordered so s-pass-1 can start early ----
            sb_up3 = res.tile([128, 49, B], BF16)
            nc.sync.dma_start(out=sb_up3, in_=up3[:])
            sb_up2 = res.tile([128, CH, I, B], BF16)
            nc.sync.dma_start(out=sb_up2, in_=up2[:])
            sb_sel = res.tile([128, I, 16], BF16)
            nc.sync.dma_start(out=sb_sel, in_=selio[:])
            sb_w2r = res.tile([128, NG, PL, I], BF16)
            w2r_flat = sb_w2r.rearrange("q g p i -> q g (p i)")
            sb_id16f = res.tile([16, 16], F32)
            nc.sync.dma_start(out=sb_id16f, in_=id16f[:])
            sb_rep16 = res.tile([16, 128], BF16)
            nc.sync.dma_start(out=sb_rep16, in_=rep16[:])
            sb_sel2 = res.tile([128, 8, 16], BF16)
            nc.sync.dma_start(out=sb_sel2, in_=selio2[:])
            sb_mask = res.tile([128, 128], BF16)
            nc.sync.dma_start(out=sb_mask, in_=bdmask[:])
            sb_id128b = res.tile([128, 128], BF16)
            nc.sync.dma_start(out=sb_id128b, in_=id128b[:])
            sb_urep0 = res.tile([128, PL, I], BF16)
            sb_urep1 = res.tile([128, PL, I], BF16)
            sb_urep = [sb_urep0, sb_urep1]

            sb_eP2 = res.tile([128, CH, N, B], BF16)
            # pad rows (p>=784 and ch6 partitions 16:128) must be finite:
            # cu multiplies them by up2's zero padding.
            nc.gpsimd.memset(sb_eP2, 0.0)
            sb_Z = res.tile([128, NB_TILES], F32)
            # iteration-1 "Z": AllReduce over 8 cores must sum to P (uniform c)
            nc.vector.memset(sb_Z, float(P) / NC)
            # s_raw columns in (g, h, ns, bs) order: col 128*(2g+h) + 16*ns + bs
            sb_sraw = res.tile([16, NG, BH, 8, 16], F32)
            sb_sglob = res.tile([16, N * B], F32)
            sb_Zg = res.tile([128, NB_TILES], F32)
            sb_vT = res.tile([16, N * B], BF16)      # squash-scaled v^T
            sb_vT8 = res.tile([128, N * B], BF16)
            sb_fac = res.tile([128, NB_TILES], F32)
            sb_ss = res.tile([128, NB_TILES], F32)
            sb_sq = res.tile([128, NB_TILES, 16], F32)   # s in NB layout
            sb_a1 = res.tile([128, NB_TILES, PL], BF16)  # iteration-1 agreements

            up2_flat = sb_up2.rearrange("q c i b -> q (c i b)")

            def warm(k):
                """Junk matmuls that keep the PE p-state ramped through
                gaps (the cost model halves the clock after any idle)."""
                for _ in range(k):
                    wj = ps_m.tile([16, 128], F32, tag="eT")
                    nc.tensor.matmul(
                        wj, up2_flat[:, 0:16], up2_flat[:, 0:128],
                        start=True, stop=True,
                    )

            pending_sel = []

            def s_jb(it, jb):
                """Partial s_raw for the 4 caps of block jb:
                s_raw[n,b,o] = sum_{p local,i} cu * W with cu = e * u
                (it=1 uses cu = u: uniform c)."""
                g_, nhalf = jb // 2, jb % 2
                wps = w2s.tile([128, CH, 4, 128], BF16, tag="wps")
                nc.sync.dma_start(
                    out=wps, in_=wp2[:, :, 4 * jb : 4 * jb + 4, :]
                )
                for half in range(2):
                    psum_s = ps_s.tile([128, 2, 256], F32, tag="ps_s")
                    for nn in range(2):
                        n = 4 * jb + 2 * half + nn
                        if it == 1:
                            cu = sb_up2
                        else:
                            cu = cupool.tile([128, CH, I, B], BF16, tag="cu")
                            e_sl = bass.AP(
                                tensor=sb_eP2.tensor,
                                offset=sb_eP2.offset + n * B,
                                ap=[sb_eP2.ap[0], [N * B, CH], [0, I], [1, B]],
                            )
                            nc.vector.tensor_tensor(
                                out=cu, in0=sb_up2, in1=e_sl, op=ALU.mult
                            )
                        cu_flat = cu.rearrange("q c i b -> q c (i b)")
                        for ch in range(CH):
                            nc.tensor.matmul(
                                psum_s[:, nn, :],
                                wps[:, ch, 2 * half + nn, :],
                                cu_flat[:, ch, :],
                                start=(ch == 0),
                                stop=(ch == CH - 1),
                            )
                    # drain psum to SBUF; the selector extraction is deferred
                    # (lag-1) so it never head-blocks the PE queue between
                    # consecutive jb matmul groups.
                    Ssb = sm.tile([128, I, 2, B], BF16, tag="S2", bufs=3)
                    nc.scalar.copy(
                        out=Ssb.rearrange("q i n b -> q n i b"),
                        in_=psum_s.rearrange("q n (i b) -> q n i b", i=I),
                    )
                    pending_sel.append((Ssb, g_, 4 * nhalf + 2 * half))
                while len(pending_sel) > 2:
                    emit_sel(*pending_sel.pop(0))

            def emit_sel(Ssb, g_, no):
                """s[o,n,b] = sum_i Ssb[(i,o), n, (i,b)] via 8 accumulating
                selector matmuls, then the sraw copy."""
                Ssb_flat = Ssb.rearrange("q i n b -> q i (n b)")
                sel_ps = ps_m.tile([16, 2, B], F32, tag="m")
                for i in range(I):
                    nc.tensor.matmul(
                        sel_ps,
                        sb_sel[:, i, :],
                        Ssb_flat[:, i, :],
                        start=(i == 0),
                        stop=(i == I - 1),
                    )
                nc.scalar.copy(
                    out=sb_sraw[:, g_, :, no : no + 2, :]
                    .rearrange("o h n b -> o n h b"),
                    in_=sel_ps.rearrange("o n (h b) -> o n h b", h=BH),
                )

            def flush_sels():
                while pending_sel:
                    emit_sel(*pending_sel.pop(0))

            def allreduce_squash(it):
                """AllReduce (s_raw ++ Z), then squash; for it<3 also emit the
                pre-scaled v^T (fac folded in) and its 8-way broadcast.
                Everything is batched across the 10 (g,h) tiles to keep this
                inter-pass bridge short."""
                cc_in = dram.tile([CCLEN], F32, tag="cc_in")
                cc_out = dram.tile([CCLEN], F32, tag="cc_out")
                nc.scalar.dma_start(out=cc_in[0 : O * N * B], in_=sb_sraw)
                nc.gpsimd.dma_start(out=cc_in[O * N * B :], in_=sb_Z)
                # dummy sqrt: pulls the Sqrt act-table load into the AR wait
                junk = sm.tile([128, 1], F32, tag="jnk")
                nc.scalar.sqrt(out=junk, in_=sb_fac[:, 0:1])
                if for_sim:
                    nc.gpsimd.dma_start(out=cc_out, in_=cc_in)
                else:
                    nc.gpsimd.collective_compute(
                        "AllReduce",
                        ALU.add,
                        replica_groups=[list(range(NC))],
                        ins=[cc_in.opt()],
                        outs=[cc_out.opt()],
                    )
                nc.scalar.dma_start(out=sb_sglob, in_=cc_out[0 : O * N * B])
                nc.gpsimd.dma_start(out=sb_Zg, in_=cc_out[O * N * B :])

                # squash runs in two batches: tiles 0-1 first (they gate
                # group 0 of the next a-pass), then tiles 2-9
                def squash_batch(t0, t1):
                    nt = t1 - t0
                    sqa_ps = ps_m.tile([128, NB_TILES, 16], F32, tag="m")
                    for t in range(t0, t1):
                        nc.tensor.transpose(
                            sqa_ps[:, t, :],
                            sb_sglob[:, 128 * t : 128 * (t + 1)],
                            sb_id16f,
                        )
                    sq2 = sm.tile([128, NB_TILES, 16], F32, tag="sq2")
                    nc.scalar.square(
                        out=sq2[:, t0:t1, :], in_=sqa_ps[:, t0:t1, :]
                    )
                    nc.scalar.copy(
                        out=sb_sq[:, t0:t1, :], in_=sqa_ps[:, t0:t1, :]
                    )
                    nc.vector.tensor_reduce(
                        out=bass.AP(
                            tensor=sb_ss.tensor,
                            offset=sb_ss.offset + t0,
                            ap=[sb_ss.ap[0], [1, nt], [1, 1]],
                        ),
                        in_=sq2[:, t0:t1, :], axis=AX.X, op=ALU.add,
                    )
                    # fac = sqrt(ss) / (Z^2 + ss)
                    z2a = sm.tile([128, NB_TILES], F32, tag="z2")
                    nc.vector.tensor_tensor(
                        out=z2a[:, t0:t1], in0=sb_Zg[:, t0:t1],
                        in1=sb_Zg[:, t0:t1], op=ALU.mult,
                    )
                    dena = sm.tile([128, NB_TILES], F32, tag="den")
                    nc.vector.tensor_tensor(
                        out=dena[:, t0:t1], in0=z2a[:, t0:t1],
                        in1=sb_ss[:, t0:t1], op=ALU.add,
                    )
                    reca = sm.tile([128, NB_TILES], F32, tag="rec")
                    nc.vector.reciprocal(out=reca[:, t0:t1], in_=dena[:, t0:t1])
                    rssa = sm.tile([128, NB_TILES], F32, tag="rss")
                    nc.scalar.sqrt(out=rssa[:, t0:t1], in_=sb_ss[:, t0:t1])
                    nc.vector.tensor_tensor(
                        out=sb_fac[:, t0:t1], in0=rssa[:, t0:t1],
                        in1=reca[:, t0:t1], op=ALU.mult,
                    )

                fac_bc0 = bass.AP(
                    tensor=sb_fac.tensor,
                    offset=sb_fac.offset,
                    ap=[sb_fac.ap[0], [1, NB_TILES], [0, 16]],
                )
                vtb = sm.tile(
                    [128, NB_TILES, 16], F32 if it == 3 else BF16, tag="vtb"
                )

                def v_batch(t0, t1):
                    fac_bc = bass.AP(
                        tensor=sb_fac.tensor,
                        offset=sb_fac.offset + t0,
                        ap=[sb_fac.ap[0], [1, t1 - t0], [0, 16]],
                    )
                    nc.vector.tensor_tensor(
                        out=vtb[:, t0:t1, :], in0=sb_sq[:, t0:t1, :],
                        in1=fac_bc, op=ALU.mult,
                    )
                    if it == 3:
                        for t in range(t0, t1):
                            g, h = t // BH, t % BH
                            nc.sync.dma_start(
                                out=vout[8 * g : 8 * g + 8,
                                         16 * h : 16 * h + 16, :],
                                in_=vtb[:, t, :],
                            )
                        return
                    # vT8[(r,o), nb] = vtb[nb, o]: per 4-tile group,
                    # transpose vtb -> vT (PSUM), drain, then one
                    # replication matmul per tile (rep16 x vT) + drain
                    for c0 in range(t0, t1, 4):
                        c1 = min(c0 + 4, t1)
                        vtT_ps = ps_m.tile([16, 4, 128], BF16, tag="eT")
                        for j in range(c1 - c0):
                            nc.tensor.transpose(
                                vtT_ps[:, j, :], vtb[:, c0 + j, :], sb_id128b
                            )
                        nc.scalar.copy(
                            out=sb_vT[:, 128 * c0 : 128 * c1],
                            in_=vtT_ps[:, : c1 - c0, :]
                            .rearrange("o t c -> o (t c)"),
                        )
                        for t in range(c0, c1):
                            v8_ps = ps_m.tile([128, 128], F32, tag="m")
                            nc.tensor.matmul(
                                v8_ps, sb_rep16,
                                sb_vT[:, 128 * t : 128 * (t + 1)],
                                start=True, stop=True,
                            )
                            nc.scalar.copy(
                                out=sb_vT8[:, 128 * t : 128 * (t + 1)],
                                in_=v8_ps,
                            )

                squash_batch(0, 2)
                v_batch(0, 2)
                squash_batch(2, 4)
                v_batch(2, 4)
                squash_batch(4, NB_TILES)
                v_batch(4, NB_TILES)
                if it == 3:
                    return
                # dummy exp: preload the Exp act-table before the a-pass
                junk2 = sm.tile([128, 1], BF16, tag="jnk2")
                nc.scalar.activation(out=junk2, in_=sb_fac[:, 0:1], func=ACTF.Exp)

            def a_group(it, g):
                """a[n,b,p] = u_hat . v for group g's two (g,h) tiles, the
                bridge into eP2 for the next s-pass, and (fused) the next
                s-pass's two jb blocks for this group's caps."""
                bds, ats = [], []
                for h in range(BH):
                    t_ = 2 * g + h
                    bd = sm.tile([128, 128], BF16, tag="bd", bufs=2)
                    nc.vector.tensor_tensor(
                        out=bd,
                        in0=sb_vT8[:, 128 * t_ : 128 * (t_ + 1)],
                        in1=sb_mask,
                        op=ALU.mult,
                    )
                    bds.append(bd)
                    if it == 1:
                        ats.append(sb_a1[:, t_, :])
                    else:
                        ats.append(atile.tile([128, PL], BF16, tag="a", name=f"at{h}"))

                # stream w2[g] in 4 pieces of up to 256 nodes, fold in
                # 128-node chunks.  Two drain routes, balanced across tiles:
                #   A: Activation copies PSUM f32 -> SBUF bf16, DVE fold at 2x
                #   B: DVE fold reads PSUM f32 directly (full rate, no Act)
                for pc in range(4):
                    pn = 256 if pc < 3 else 16
                    off = 256 * pc
                    for h in range(BH):
                        t_ = 2 * g + h
                        route_b = t_ in ROUTE_B_TILES
                        for sub in range(2 if pc < 3 else 1):
                            cn = 128 if pc < 3 else 16
                            coff = off + 128 * sub
                            wv_ps = ps_wv.tile([128, 128, I], F32, tag="wv")
                            wv_flat = wv_ps.rearrange("q p i -> q (p i)")
                            for sck in range(2 if pc < 3 else 1):
                                F = (64 if pc < 3 else 16) * I
                                nc.tensor.matmul(
                                    wv_flat[:, 512 * sck : 512 * sck + F],
                                    bds[h],
                                    w2r_flat[
                                        :, g,
                                        I * coff + 512 * sck :
                                        I * coff + 512 * sck + F,
                                    ],
                                    start=True,
                                    stop=True,
                                )
                            warm(W_A)
                            if route_b:
                                wv_in = wv_ps
                            else:
                                wv_sb = work.tile([128, 128, I], BF16, tag="wvs")
                                nc.scalar.copy(
                                    out=wv_sb[:, :cn, :], in_=wv_ps[:, :cn, :]
                                )
                                wv_in = wv_sb
                            ts_ = work.tile([128, 128, I], BF16, tag="ts")
                            nc.vector.tensor_tensor(
                                out=ts_[:, :cn, :],
                                in0=wv_in[:, :cn, :],
                                in1=sb_urep[h][:, coff : coff + cn, :],
                                op=ALU.mult,
                            )
                            r1 = work.tile([128, 128, 4], BF16, tag="r1")
                            if t_ in R1_POOL_TILES:
                                nc.gpsimd.scalar_tensor_tensor(
                                    out=r1[:, :cn, :], in0=ts_[:, :cn, 0:4],
                                    scalar=1.0, in1=ts_[:, :cn, 4:8],
                                    op0=ALU.mult, op1=ALU.add,
                                )
                            else:
                                nc.vector.tensor_tensor(
                                    out=r1[:, :cn, :], in0=ts_[:, :cn, 0:4],
                                    in1=ts_[:, :cn, 4:8], op=ALU.add,
                                )
                            r2 = work.tile([128, 128, 2], BF16, tag="r2")
                            if R2_POOL:
                                nc.gpsimd.scalar_tensor_tensor(
                                    out=r2[:, :cn, :], in0=r1[:, :cn, 0:2],
                                    scalar=1.0, in1=r1[:, :cn, 2:4],
                                    op0=ALU.mult, op1=ALU.add,
                                )
                                nc.gpsimd.scalar_tensor_tensor(
                                    out=ats[h][:, coff : coff + cn],
                                    in0=r2[:, :cn, 0], scalar=1.0,
                                    in1=r2[:, :cn, 1],
                                    op0=ALU.mult, op1=ALU.add,
                                )
                            else:
                                nc.vector.tensor_tensor(
                                    out=r2[:, :cn, :], in0=r1[:, :cn, 0:2],
                                    in1=r1[:, :cn, 2:4], op=ALU.add,
                                )
                                nc.vector.tensor_tensor(
                                    out=ats[h][:, coff : coff + cn],
                                    in0=r2[:, :cn, 0], in1=r2[:, :cn, 1],
                                    op=ALU.add,
                                )

                return ats

            def a_bridge(it, g, ats):
                # ---- bridge to next s-pass: exp + transpose into eP2 ----
                for h in range(BH):
                    t = 2 * g + h
                    if it == 1:
                        bt = sb_a1[:, t, :]
                    else:
                        bt = atile.tile([128, PL], BF16, tag="b2", bufs=2)
                        nc.vector.tensor_tensor(
                            out=bt, in0=ats[h], in1=sb_a1[:, t, :], op=ALU.add
                        )
                    e_nb = work.tile([128, PL], BF16, tag="enb")
                    nc.scalar.activation(
                        out=e_nb, in_=bt, func=ACTF.Exp,
                        accum_out=sb_Z[:, t : t + 1],
                    )
                    # 7 PE transposes batched into two PSUM tiles, three
                    # Activation copies (ch6 separate: only 16 valid rows).
                    for cg, nch in ((0, 4), (4, 2)):
                        eT_ps = ps_m.tile([128, 4, 128], BF16, tag="eT")
                        for j in range(nch):
                            ch = cg + j
                            nc.tensor.transpose(
                                eT_ps[:, j, :],
                                e_nb[:, 128 * ch : 128 * (ch + 1)],
                                sb_id128b,
                            )
                        nc.scalar.copy(
                            out=sb_eP2[:, cg : cg + nch, 8 * g : 8 * g + 8,
                                       16 * h : 16 * h + 16],
                            in_=eT_ps[:, :nch, :]
                            .rearrange("p c (n b) -> p c n b", n=8),
                        )
                    eT6 = ps_m.tile([128, 4, 128], BF16, tag="eT")
                    nc.tensor.transpose(
                        eT6[:16, 0, :], e_nb[:, 768 : 768 + 16], sb_id128b
                    )
                    nc.scalar.copy(
                        out=sb_eP2[:16, 6, 8 * g : 8 * g + 8,
                                   16 * h : 16 * h + 16],
                        in_=eT6[:16, 0, :].rearrange("p (n b) -> p n b", n=8),
                    )



            # ---- schedule ----
            # s-pass 1: c is uniform, so the rhs (u) is shared across all
            # capsules -- contract (p,i) jointly with (n,o)-batched weights:
            # 49 accumulating 32-wide matmuls per n-group.
            ps1 = ps_s.tile([128, NG, B], F32, tag="ps_s")
            for g_ in range(NG):
                for half_ in range(2):
                    c0, c1 = (0, 25) if half_ == 0 else (25, 49)
                    wqp = w2s.tile([128, 25, 128], BF16, tag="wps")
                    nc.sync.dma_start(
                        out=wqp[:, : c1 - c0, :], in_=wq[g_, :, c0:c1, :]
                    )
                    for c_ in range(c0, c1):
                        nc.tensor.matmul(
                            ps1[:, g_, :],
                            wqp[:, c_ - c0, :],
                            sb_up3[:, c_, :],
                            start=(c_ == 0),
                            stop=(c_ == 48),
                        )
                pg = sm.tile([128, B], BF16, tag="pg", bufs=2)
                nc.scalar.copy(out=pg, in_=ps1[:, g_, :])
                for j_ in range(4):
                    sel_ps = ps_m.tile([16, 2, B], F32, tag="m")
                    for nn_ in range(2):
                        nc.tensor.matmul(
                            sel_ps[:, nn_, :],
                            sb_sel2[:, 2 * j_ + nn_, :],
                            pg,
                            start=True,
                            stop=True,
                        )
                    nc.scalar.copy(
                        out=sb_sraw[:, g_, :, 2 * j_ : 2 * j_ + 2, :]
                        .rearrange("o h n b -> o n h b"),
                        in_=sel_ps.rearrange("o n (h b) -> o n h b", h=BH),
                    )
            for q_ in range(4):
                nc.sync.dma_start(
                    out=sb_w2r[:, 0, 196 * q_ : 196 * (q_ + 1), :],
                    in_=w2[0, :, 196 * q_ : 196 * (q_ + 1), :],
                )
            for q_ in range(2):
                nc.sync.dma_start(
                    out=sb_urep0[:, 392 * q_ : 392 * (q_ + 1), :],
                    in_=urep[0, :, 392 * q_ : 392 * (q_ + 1), :],
                )
                nc.sync.dma_start(
                    out=sb_urep1[:, 392 * q_ : 392 * (q_ + 1), :],
                    in_=urep[1, :, 392 * q_ : 392 * (q_ + 1), :],
                )
            flush_sels()
            allreduce_squash(1)
            for g_ in range(1, NG):
                for q_ in range(4):
                    nc.sync.dma_start(
                        out=sb_w2r[:, g_, 196 * q_ : 196 * (q_ + 1), :],
                        in_=w2[g_, :, 196 * q_ : 196 * (q_ + 1), :],
                    )
            prev = None
            for g in range(NG):
                ats_g = a_group(1, g)
                if prev is not None:
                    a_bridge(1, g - 1, prev)
                    if g >= 2:
                        s_jb(2, 2 * (g - 2))
                        s_jb(2, 2 * (g - 2) + 1)
                prev = ats_g
            a_bridge(1, NG - 1, prev)
            for jb in (2 * (NG - 2), 2 * (NG - 2) + 1, 2 * (NG - 1),
                       2 * (NG - 1) + 1):
                s_jb(2, jb)
            flush_sels()
            allreduce_squash(2)
            prev = None
            for g in range(NG):
                ats_g = a_group(2, g)
                if prev is not None:
                    a_bridge(2, g - 1, prev)
                    if g >= 2:
                        s_jb(3, 2 * (g - 2))
                        s_jb(3, 2 * (g - 2) + 1)
                prev = ats_g
            a_bridge(2, NG - 1, prev)
            for jb in (2 * (NG - 2), 2 * (NG - 2) + 1, 2 * (NG - 1),
                       2 * (NG - 1) + 1):
                s_jb(3, jb)
            flush_sels()
            allreduce_squash(3)

    nc.finalize()
    return nc


_CACHE = {}


def _prep_inputs(u, W):
    """Per-core host-side relayout (not part of HW time)."""
    bf = ml_dtypes.bfloat16
    maps = []
    for c in range(NC):
        sl = slice(PL * c, PL * (c + 1))
        Wc = np.ascontiguousarray(W[:, sl])          # [40, 784, 8, 16] f32
        uc = np.ascontiguousarray(u[:, sl])          # [32, 784, 8] f32
        w2 = (
            Wc.reshape(NG, 8, PL, I, O)
            .transpose(0, 1, 4, 2, 3)
            .reshape(NG, 128, PL, I)
        )
        Wp = np.zeros((N, PPAD, I, O), np.float32)
        Wp[:, :PL] = Wc
        wp2 = Wp.reshape(N, CH, 128, 128).transpose(2, 1, 0, 3)
        Up = np.zeros((B, PPAD, I), np.float32)
        Up[:, :PL] = uc
        up2 = Up.reshape(B, CH, 128, I).transpose(2, 1, 3, 0)
        ur = np.broadcast_to(
            uc.reshape(1, BH, 16, PL, I), (8, BH, 16, PL, I)
        ).transpose(1, 0, 2, 3, 4).reshape(BH, 128, PL, I)
        bdm = np.zeros((128, 128), np.float32)
        for ns in range(8):
            bdm[ns * 16 : ns * 16 + 16, ns * 16 : ns * 16 + 16] = 1.0
        sel = np.zeros((128, I, 16), np.float32)
        for i in range(I):
            sel[16 * i : 16 * i + 16, i] = np.eye(16, dtype=np.float32)
        # wq[g, (pp,i), c, (n8,o)] = Wc[8g+n8, 16c+pp, i, o]
        wqh = (
            Wc.reshape(NG, 8, 49, 16, I, O)
            .transpose(0, 3, 4, 2, 1, 5)
            .reshape(NG, 128, 49, 128)
        )
        # up3[(pp,i), c, b] = uc[b, 16c+pp, i]
        u3 = (
            uc.reshape(B, 49, 16, I)
            .transpose(2, 3, 1, 0)
            .reshape(128, 49, B)
        )
        sel2 = np.zeros((128, 8, 16), np.float32)
        for cap in range(8):
            for o in range(16):
                sel2[16 * cap + o, cap, o] = 1.0
        maps.append(
            {
                "w2": np.ascontiguousarray(w2).astype(bf),
                "wp2": np.ascontiguousarray(wp2).astype(bf),
                "up2": np.ascontiguousarray(up2).astype(bf),
                "urep": np.ascontiguousarray(ur).astype(bf),
                "bdmask": bdm.astype(bf),
                "selio": sel.astype(bf),
                "id128b": np.eye(128, dtype=np.float32).astype(bf),
                "id16f": np.eye(16, dtype=np.float32),
                "rep16": np.tile(np.eye(16, dtype=np.float32), (1, 8)).astype(bf),
                "wq": np.ascontiguousarray(wqh).astype(bf),
                "up3": np.ascontiguousarray(u3).astype(bf),
                "selio2": sel2.astype(bf),
            }
        )
    return maps


def kernel(u, W):
    u = np.asarray(u, np.float32)
    W = np.asarray(W, np.float32)
    if "nc" not in _CACHE:
        _CACHE["nc"] = _build_program()
    nc = _CACHE["nc"]
    in_maps = _prep_inputs(u, W)
    res = run_bass_kernel_spmd(
        nc, in_maps, core_ids=list(range(NC)),
        trace=bool(int(os.environ.get("KERNEL_TRACE", "0"))),
    )
    _CACHE["last_result"] = res
    return res.results[0]["vout"]
